# revision 17
# baseline (speedup 1.0000x reference)
"""Head-parallel distributed MHA forward for 8 TRN2 NeuronCores.

Problem: y = proj(softmax((x Wq^T + bq)(x Wk^T + bk)^T / sqrt(hd)) (x Wv^T + bv))
  x: [4, 2048, 1024], 16 heads, head_dim 64, fp32.

Sharding: tensor-parallel over heads with host-side reduce. Core i owns
heads {2i, 2i+1} (its contiguous 128 qkv dims). Every core receives the
FULL feature-major activations xT [1024, 8192] and computes q/k/v for all
8192 tokens but only its own 2 heads (1/8 of the projection FLOPs, no
redundancy). Attention per (batch, 512-query-chunk):
  - QK^T: both heads via two concurrent row-group matmuls (head A dims on
    partitions 0:64, head B on 64:128 -> tile_position (0,0)/(64,0)).
  - exp on ScalarE (fused *scale) -> bf16 scores in SBUF. This is the
    critical path: 256 x ~1.1us activations.
  - AV with the ones-column trick: stationary [v_h | 1] (65 cols), so the
    matmul accumulates both the attention output and the softmax
    denominator Z; normalization = reciprocal + gpsimd partition
    broadcast + one DVE multiply per head, immediately per chunk.
Output projection is PARTIAL: y_i^T = Wp[:, my 128 dims]^T ot  (full 1024
out dims x 8192 tokens, bf16). The host sums the 8 partials and adds the
bias (b_v folds in: (o + bv) Wp^T + bp = o Wp^T + (Wp bv + bp)), so no
device collectives are needed.

Weights are host-reformatted into SBUF layout ([128 partitions, 1024]
with 2KB contiguous rows) so each loads as one large-packet DMA; the
4-byte-per-packet bias transfer is packed into one [128, 2] tensor.

Schedule: attention starts as soon as k/q for the first 512 tokens and v
for the first chunk exist (~12us); all remaining q/k/v units and the
partial-proj units are drip-fed into the PE stream with a credit budget
per k-tile so the in-order PE queue neither bursts ahead of nor starves
the ScalarE exp stream.
"""

import numpy as np

P = 128
D = 1024
NH = 16
HD = 64
SCALE = 1.0 / float(np.sqrt(HD))
NCORES = 8
B, T = 4, 2048
NTOK = B * T           # 8192
QCH = 512              # query tokens per attention group
NQC = T // QCH         # 4 query chunks per batch
NKT = T // P           # 16 k-tiles per batch
CCH = 1024             # projection token chunk
NCH = NTOK // CCH      # 8 chunks
NFT = D // P           # 8 contraction tiles
VW = 66                # per-head stationary stride in vt (v + ones + pad)

_COMPILED = {}


def build():
    from concourse import bacc
    import concourse.mybir as mybir
    import concourse.tile as tile

    f32 = mybir.dt.float32
    bf16 = mybir.dt.bfloat16
    AF = mybir.ActivationFunctionType

    nc = bacc.Bacc("TRN2", target_bir_lowering=False, debug=False,
                   num_devices=NCORES)

    xT = nc.dram_tensor("xT", [D, NTOK], bf16, kind="ExternalInput")
    # weights pre-arranged in SBUF layout [128, 8*128]
    w_q = nc.dram_tensor("w_q", [P, D], bf16, kind="ExternalInput")
    w_k = nc.dram_tensor("w_k", [P, D], bf16, kind="ExternalInput")
    w_v = nc.dram_tensor("w_v", [P, D], bf16, kind="ExternalInput")
    w_p = nc.dram_tensor("w_p", [P, D], bf16, kind="ExternalInput")
    b_qk = nc.dram_tensor("b_qk", [P, 2], f32, kind="ExternalInput")
    outT = nc.dram_tensor("out", [D, NTOK], bf16, kind="ExternalOutput")

    with tile.TileContext(nc) as tc:
        with (
            tc.tile_pool(name="persist", bufs=1) as persist,
            tc.tile_pool(name="xpool", bufs=3) as xpool,
            tc.tile_pool(name="ptpool", bufs=8) as ptpool,
            tc.tile_pool(name="zpool", bufs=2) as zpool,
            tc.tile_pool(name="rzbp", bufs=2) as rzbp,
            tc.tile_pool(name="ypool", bufs=3) as ypool,
            tc.tile_pool(name="psmm", bufs=2, space="PSUM") as psmm,
            tc.tile_pool(name="pst", bufs=2, space="PSUM") as pst,
            tc.tile_pool(name="pot", bufs=2, space="PSUM") as pot,
        ):
            # ---- persistent SBUF ----
            wq_sb = persist.tile([P, NFT, P], bf16)
            wk_sb = persist.tile([P, NFT, P], bf16)
            wv_sb = persist.tile([P, NFT, P], bf16)
            wp_sb = persist.tile([P, NFT, P], bf16)
            bqk_sb = persist.tile([P, 2], f32)

            q_all = persist.tile([P, NTOK], bf16)    # [head dims A|B, tok]
            kt_all = persist.tile([P, NTOK], bf16)   # [head dims A|B, tok]
            # [tok, ktile, vA(64) 1 pad vB(64) 1 pad]
            vt_all = persist.tile([P, NTOK // P, 2 * VW], bf16)
            ot_all = persist.tile([P, NTOK], bf16)   # normalized attn out^T

            xh = {}

            def load_x(c):
                x_sb = xpool.tile([P, NFT, CCH], bf16, tag="x", name="x_sb")
                for dc in range(NFT):
                    nc.sync.dma_start(
                        x_sb[:, dc, :],
                        xT[dc * P:(dc + 1) * P, c * CCH:(c + 1) * CCH])
                xh[c] = x_sb

            # DMA order: x(0) first (critical path), then wk/biases, x(1),
            # remaining weights; each weight is one big-packet DMA.
            load_x(0)
            nc.sync.dma_start(wk_sb[:], w_k[:])
            nc.sync.dma_start(bqk_sb[:], b_qk[:])
            nc.sync.dma_start(wq_sb[:], w_q[:])
            nc.sync.dma_start(wv_sb[:], w_v[:])
            load_x(1)
            nc.sync.dma_start(wp_sb[:], w_p[:])

            nc.vector.memset(vt_all[:, :, HD], 1.0)          # ones col A
            nc.vector.memset(vt_all[:, :, VW + HD], 1.0)     # ones col B

            def qk_unit(c, h, w_sb, bcol, dst):
                """q or k for 512 tokens of chunk c -> dst[:, tok slice]."""
                t0 = c * CCH + h * QCH
                ps = psmm.tile([P, QCH], f32, tag="mm", name="ps_qk")
                for dc in range(NFT):
                    nc.tensor.matmul(
                        ps[:], w_sb[:, dc, :],
                        xh[c][:, dc, h * QCH:(h + 1) * QCH],
                        start=(dc == 0), stop=(dc == NFT - 1))
                nc.vector.tensor_scalar_add(
                    dst[:, t0:t0 + QCH], ps[:], bqk_sb[:, bcol:bcol + 1])

            def v_unit(c, tt):
                """v for 128 tokens (tile c*8+tt), both heads' 128 dims."""
                kti = c * (CCH // P) + tt
                ps = psmm.tile([P, QCH], f32, tag="mm", name="ps_v")
                for dc in range(NFT):
                    nc.tensor.matmul(
                        ps[:, 0:P],
                        xh[c][:, dc, tt * P:(tt + 1) * P],
                        wv_sb[:, dc, :],
                        start=(dc == 0), stop=(dc == NFT - 1))
                nc.vector.tensor_copy(vt_all[:, kti, 0:HD], ps[:, 0:HD])
                nc.vector.tensor_copy(
                    vt_all[:, kti, VW:VW + HD], ps[:, HD:P])

            def proj_unit(b, qc, jt):
                """Partial out-proj for 512 tokens, 128 output dims."""
                t0 = b * T + qc * QCH
                ps = psmm.tile([P, QCH], f32, tag="mm", name="ps_p")
                nc.tensor.matmul(
                    ps[:], wp_sb[:, jt, :], ot_all[:, t0:t0 + QCH],
                    start=True, stop=True)
                ysb = ypool.tile([P, QCH], bf16, name="ysb")
                if b == B - 1 and qc >= NQC - 2:
                    nc.scalar.copy(ysb[:], ps[:])   # ScalarE idle post-exp
                else:
                    nc.vector.tensor_copy(ysb[:], ps[:])
                nc.sync.dma_start(
                    outT[jt * P:(jt + 1) * P, t0:t0 + QCH], ysb[:])

            # ---- filler machinery: drip units into the PE stream with a
            # per-k-tile credit budget (cost ~ matmul count)
            chunk_q = []   # (chunk_id, cost, thunk) -- deadline-ordered
            proj_q = []
            credit = [0.0]

            def emit_fill(budget):
                credit[0] += budget
                while True:
                    if chunk_q:
                        cid, cost, u = chunk_q[0]
                    elif proj_q:
                        cid, cost, u = 0, 1, None
                    else:
                        return
                    if cost > credit[0]:
                        return
                    credit[0] -= cost
                    if u is None:
                        proj_q.pop(0)()
                    else:
                        chunk_q.pop(0)
                        u()

            def force_chunks(upto):
                while chunk_q and chunk_q[0][0] < upto:
                    _, _, u = chunk_q.pop(0)
                    u()

            def attn(b, qc):
                t0 = b * T + qc * QCH
                otA = pot.tile([P, QCH], f32, tag="ot", name="otA")
                otB = pot.tile([P, QCH], f32, tag="ot", name="otB")
                pend = []

                def emit_av(kt, pt):
                    kti = b * NKT + kt
                    nc.tensor.matmul(
                        otA[0:HD + 1, :], vt_all[:, kti, 0:HD + 1],
                        pt[:, 0:QCH],
                        start=(kt == 0), stop=(kt == NKT - 1))
                    nc.tensor.matmul(
                        otB[0:HD + 1, :], vt_all[:, kti, VW:VW + HD + 1],
                        pt[:, QCH:2 * QCH],
                        start=(kt == 0), stop=(kt == NKT - 1))

                for kt in range(NKT):
                    k0 = b * T + kt * P
                    st = pst.tile([P, 2 * QCH], f32, tag="st", name="st")
                    nc.tensor.matmul(
                        st[:, 0:QCH], kt_all[0:HD, k0:k0 + P],
                        q_all[0:HD, t0:t0 + QCH], start=True, stop=True)
                    nc.tensor.matmul(
                        st[:, QCH:2 * QCH], kt_all[HD:P, k0:k0 + P],
                        q_all[HD:P, t0:t0 + QCH], start=True, stop=True)
                    pt = ptpool.tile([P, 2 * QCH], bf16, tag="pt", name="pt")
                    nc.scalar.activation(pt[:], st[:], AF.Exp, scale=SCALE)
                    pend.append((kt, pt))
                    if len(pend) > 4:
                        emit_av(*pend.pop(0))
                    emit_fill(4)
                if b == 0 and qc == 0:
                    force_chunks(2)   # chunk-1 k/v must precede the flush
                for pe_ in pend:
                    emit_av(*pe_)
                emit_fill(6)

                # normalize both heads into ot_all (Z sits at ot[HD])
                for hh, ot in ((0, otA), (1, otB)):
                    zrow = zpool.tile([1, QCH], f32, tag="z", name="zrow",
                                      bufs=1)
                    nc.vector.tensor_copy(zrow[:], ot[HD:HD + 1, :])
                    rz = zpool.tile([1, QCH], f32, tag="rz", name="rz")
                    nc.vector.reciprocal_approx_fast(rz[:], zrow[:])
                    rb = rzbp.tile([HD, QCH], f32, tag="rzb", name="rb")
                    nc.gpsimd.partition_broadcast(rb[:], rz[:])
                    nc.vector.tensor_mul(
                        ot_all[hh * HD:(hh + 1) * HD, t0:t0 + QCH],
                        ot[0:HD, :], rb[:])

            # ---- schedule ----
            # Minimal serial prefix before attention: k/q/v of chunk 0 only
            # (group (0,0) ktiles 0-7).  Chunk 1's k and v and the later q
            # units drip in as the FIRST fills, ordered so each lands a few
            # ktiles before its first consumer (k(c1) by kt 8, v(c1) tile j
            # by AV kt 8+j which trails emission by 4-5 ktiles, q(c0,h1) by
            # group (0,1), q(c1,*) by groups (0,2)/(0,3)).
            for h in range(CCH // QCH):
                qk_unit(0, h, wk_sb, 1, kt_all)
            qk_unit(0, 0, wq_sb, 0, q_all)
            for tt in range(CCH // P):
                v_unit(0, tt)
            for h in range(CCH // QCH):
                chunk_q.append(
                    (1, 8, lambda h=h: qk_unit(1, h, wk_sb, 1, kt_all)))
            for tt in range(CCH // P // 2):
                chunk_q.append((1, 8, lambda tt=tt: v_unit(1, tt)))
            chunk_q.append((0, 8, lambda: qk_unit(0, 1, wq_sb, 0, q_all)))
            for tt in range(CCH // P // 2, CCH // P):
                chunk_q.append((1, 8, lambda tt=tt: v_unit(1, tt)))
            chunk_q.append((1, 8, lambda: qk_unit(1, 0, wq_sb, 0, q_all)))
            chunk_q.append((1, 8, lambda: qk_unit(1, 1, wq_sb, 0, q_all)))

            for c in range(2, NCH):
                chunk_q.append((c, 0, lambda c=c: load_x(c)))
                for h in range(CCH // QCH):
                    chunk_q.append(
                        (c, 8,
                         lambda c=c, h=h: qk_unit(c, h, wk_sb, 1, kt_all)))
                for tt in range(CCH // P):
                    chunk_q.append((c, 8, lambda c=c, tt=tt: v_unit(c, tt)))
                for h in range(CCH // QCH):
                    chunk_q.append(
                        (c, 8,
                         lambda c=c, h=h: qk_unit(c, h, wq_sb, 0, q_all)))

            for b in range(B):
                if b >= 1:
                    force_chunks(2 * (b + 1) if b < B - 1 else NCH)
                for qc in range(NQC):
                    attn(b, qc)
                    for jt in range(NFT):
                        proj_q.append(
                            lambda b=b, qc=qc, jt=jt: proj_unit(b, qc, jt))
                    emit_fill(4)
            while chunk_q or proj_q:
                emit_fill(8)

    nc.compile()
    return nc


def make_in_maps(inputs):
    """Host-side sharding: full inputs -> per-core input dicts."""
    import ml_dtypes
    bf = ml_dtypes.bfloat16

    x = np.asarray(inputs["x"], dtype=np.float32)
    w_qkv = np.asarray(inputs["w_qkv"], dtype=np.float32)
    b_qkv = np.asarray(inputs["b_qkv"], dtype=np.float32)
    w_proj = np.asarray(inputs["w_proj"], dtype=np.float32)

    def sbuf_layout(a):
        # [1024 x-dims, 128 my-dims] -> [128 part, 8*128] SBUF image
        return np.ascontiguousarray(
            a.reshape(NFT, P, P).transpose(1, 0, 2).reshape(P, D)).astype(bf)

    xT = np.ascontiguousarray(x.reshape(NTOK, D).T).astype(bf)
    in_maps = []
    for i in range(NCORES):
        s = slice(P * i, P * (i + 1))
        in_maps.append({
            "xT": xT,
            "w_q": sbuf_layout(w_qkv[0:D][s].T),
            "w_k": sbuf_layout(w_qkv[D:2 * D][s].T),
            "w_v": sbuf_layout(w_qkv[2 * D:3 * D][s].T),
            "w_p": np.ascontiguousarray(w_proj[:, s].T).astype(bf),
            "b_qk": np.ascontiguousarray(
                np.stack([b_qkv[0:D][s], b_qkv[D:2 * D][s]], axis=1)),
        })
    return in_maps


def assemble_output(results, inputs):
    x = np.asarray(inputs["x"])
    w_proj = np.asarray(inputs["w_proj"], dtype=np.float64)
    b_qkv = np.asarray(inputs["b_qkv"], dtype=np.float64)
    b_proj = np.asarray(inputs["b_proj"], dtype=np.float64)
    b_eff = b_proj + w_proj @ b_qkv[2 * D:3 * D]

    acc = np.zeros((D, NTOK), dtype=np.float32)
    for i in range(NCORES):
        acc += np.asarray(results[i]["out"], dtype=np.float32)
    y = acc.T.astype(np.float64) + b_eff[None, :]
    return y.reshape(x.shape).astype(np.float32)


def run(inputs, trace=False, **kw):
    from concourse.bass_utils import run_bass_kernel_spmd
    key = "full"
    if key not in _COMPILED:
        _COMPILED[key] = build()
    nc = _COMPILED[key]
    in_maps = make_in_maps(inputs)
    res = run_bass_kernel_spmd(nc, in_maps, core_ids=list(range(NCORES)),
                               trace=trace, **kw)
    return res


def kernel(**inputs) -> np.ndarray:
    res = run(inputs, trace=False)
    return assemble_output(res.results, inputs)



# revision 18
# speedup vs baseline: 1.1909x; 1.1909x over previous
"""Head-parallel distributed MHA forward for 8 TRN2 NeuronCores.

Problem: y = proj(softmax((x Wq^T + bq)(x Wk^T + bk)^T / sqrt(hd)) (x Wv^T + bv))
  x: [4, 2048, 1024], 16 heads, head_dim 64, fp32.

Sharding: tensor-parallel over heads with host-side reduce. Core i owns
heads {2i, 2i+1} (its contiguous 128 qkv dims). Every core receives the
FULL feature-major activations xT [1024, 8192] and computes q/k/v for all
8192 tokens but only its own 2 heads (1/8 of the projection FLOPs, no
redundancy). Attention per (batch, 512-query-chunk):
  - QK^T: both heads via two concurrent row-group matmuls (head A dims on
    partitions 0:64, head B on 64:128 -> tile_position (0,0)/(64,0)).
  - exp on ScalarE (fused *scale) -> bf16 scores in SBUF. This is the
    critical path: 256 x ~1.1us activations.
  - AV with the ones-column trick: stationary [v_h | 1] (65 cols), so the
    matmul accumulates both the attention output and the softmax
    denominator Z; normalization = reciprocal + gpsimd partition
    broadcast + one DVE multiply per head, immediately per chunk.
Output projection is PARTIAL: y_i^T = Wp[:, my 128 dims]^T ot  (full 1024
out dims x 8192 tokens, bf16). The host sums the 8 partials and adds the
bias (b_v folds in: (o + bv) Wp^T + bp = o Wp^T + (Wp bv + bp)), so no
device collectives are needed.

Weights are host-reformatted into SBUF layout ([128 partitions, 1024]
with 2KB contiguous rows) so each loads as one large-packet DMA; the
4-byte-per-packet bias transfer is packed into one [128, 2] tensor.

Schedule: attention starts as soon as k/q for the first 512 tokens and v
for the first chunk exist (~12us); all remaining q/k/v units and the
partial-proj units are drip-fed into the PE stream with a credit budget
per k-tile so the in-order PE queue neither bursts ahead of nor starves
the ScalarE exp stream.
"""

import numpy as np

P = 128
D = 1024
NH = 16
HD = 64
SCALE = 1.0 / float(np.sqrt(HD))
NCORES = 8
B, T = 4, 2048
NTOK = B * T           # 8192
QCH = 512              # query tokens per attention group
NQC = T // QCH         # 4 query chunks per batch
NKT = T // P           # 16 k-tiles per batch
CCH = 1024             # projection token chunk
NCH = NTOK // CCH      # 8 chunks
NFT = D // P           # 8 contraction tiles
VW = 66                # per-head stationary stride in vt (v + ones + pad)

_COMPILED = {}


def build():
    from concourse import bacc
    import concourse.mybir as mybir
    import concourse.tile as tile

    f32 = mybir.dt.float32
    bf16 = mybir.dt.bfloat16
    AF = mybir.ActivationFunctionType

    nc = bacc.Bacc("TRN2", target_bir_lowering=False, debug=False,
                   num_devices=NCORES)

    xT = nc.dram_tensor("xT", [D, NTOK], bf16, kind="ExternalInput")
    # weights pre-arranged in SBUF layout [128, 8*128]
    w_q = nc.dram_tensor("w_q", [P, D], bf16, kind="ExternalInput")
    w_k = nc.dram_tensor("w_k", [P, D], bf16, kind="ExternalInput")
    w_v = nc.dram_tensor("w_v", [P, D], bf16, kind="ExternalInput")
    w_p = nc.dram_tensor("w_p", [P, D], bf16, kind="ExternalInput")
    b_qk = nc.dram_tensor("b_qk", [P, 2], f32, kind="ExternalInput")
    outT = nc.dram_tensor("out", [D, NTOK], bf16, kind="ExternalOutput")

    with tile.TileContext(nc) as tc:
        with (
            tc.tile_pool(name="persist", bufs=1) as persist,
            tc.tile_pool(name="xpool", bufs=3) as xpool,
            tc.tile_pool(name="ptpool", bufs=8) as ptpool,
            tc.tile_pool(name="zpool", bufs=2) as zpool,
            tc.tile_pool(name="rzbp", bufs=2) as rzbp,
            tc.tile_pool(name="ypool", bufs=3) as ypool,
            tc.tile_pool(name="psmm", bufs=2, space="PSUM") as psmm,
            tc.tile_pool(name="pst", bufs=2, space="PSUM") as pst,
            tc.tile_pool(name="pot", bufs=2, space="PSUM") as pot,
        ):
            # ---- persistent SBUF ----
            wq_sb = persist.tile([P, NFT, P], bf16)
            wk_sb = persist.tile([P, NFT, P], bf16)
            wv_sb = persist.tile([P, NFT, P], bf16)
            wp_sb = persist.tile([P, NFT, P], bf16)
            bqk_sb = persist.tile([P, 2], f32)

            q_all = persist.tile([P, NTOK], bf16)    # [head dims A|B, tok]
            kt_all = persist.tile([P, NTOK], bf16)   # [head dims A|B, tok]
            # [tok, ktile, vA(64) 1 pad vB(64) 1 pad]
            vt_all = persist.tile([P, NTOK // P, 2 * VW], bf16)
            ot_all = persist.tile([P, NTOK], bf16)   # normalized attn out^T

            xh = {}

            def load_x(c):
                x_sb = xpool.tile([P, NFT, CCH], bf16, tag="x", name="x_sb")
                for dc in range(NFT):
                    nc.sync.dma_start(
                        x_sb[:, dc, :],
                        xT[dc * P:(dc + 1) * P, c * CCH:(c + 1) * CCH])
                xh[c] = x_sb

            # DMA order: the small weight tensors FIRST — the sync queue
            # is serial, and the first k-unit matmul needs wk + only the
            # first x tile; queueing wk behind the full 2MB x transfer
            # delayed the first matmul by ~5us.
            nc.sync.dma_start(wk_sb[:], w_k[:])
            nc.sync.dma_start(bqk_sb[:], b_qk[:])
            nc.sync.dma_start(wq_sb[:], w_q[:])
            load_x(0)
            nc.sync.dma_start(wv_sb[:], w_v[:])
            load_x(1)
            nc.sync.dma_start(wp_sb[:], w_p[:])

            nc.vector.memset(vt_all[:, :, HD], 1.0)          # ones col A
            nc.vector.memset(vt_all[:, :, VW + HD], 1.0)     # ones col B

            def qk_unit(c, h, w_sb, bcol, dst):
                """q or k for 512 tokens of chunk c -> dst[:, tok slice]."""
                t0 = c * CCH + h * QCH
                ps = psmm.tile([P, QCH], f32, tag="mm", name="ps_qk")
                for dc in range(NFT):
                    nc.tensor.matmul(
                        ps[:], w_sb[:, dc, :],
                        xh[c][:, dc, h * QCH:(h + 1) * QCH],
                        start=(dc == 0), stop=(dc == NFT - 1))
                nc.vector.tensor_scalar_add(
                    dst[:, t0:t0 + QCH], ps[:], bqk_sb[:, bcol:bcol + 1])

            def v_unit(c, tt):
                """v for 128 tokens (tile c*8+tt), both heads' 128 dims."""
                kti = c * (CCH // P) + tt
                ps = psmm.tile([P, QCH], f32, tag="mm", name="ps_v")
                for dc in range(NFT):
                    nc.tensor.matmul(
                        ps[:, 0:P],
                        xh[c][:, dc, tt * P:(tt + 1) * P],
                        wv_sb[:, dc, :],
                        start=(dc == 0), stop=(dc == NFT - 1))
                nc.vector.tensor_copy(vt_all[:, kti, 0:HD], ps[:, 0:HD])
                nc.vector.tensor_copy(
                    vt_all[:, kti, VW:VW + HD], ps[:, HD:P])

            def proj_unit(b, qc, jt):
                """Partial out-proj for 512 tokens, 128 output dims."""
                t0 = b * T + qc * QCH
                ps = psmm.tile([P, QCH], f32, tag="mm", name="ps_p")
                nc.tensor.matmul(
                    ps[:], wp_sb[:, jt, :], ot_all[:, t0:t0 + QCH],
                    start=True, stop=True)
                ysb = ypool.tile([P, QCH], bf16, name="ysb")
                if b == B - 1 and qc >= NQC - 2:
                    nc.scalar.copy(ysb[:], ps[:])   # ScalarE idle post-exp
                else:
                    nc.vector.tensor_copy(ysb[:], ps[:])
                nc.sync.dma_start(
                    outT[jt * P:(jt + 1) * P, t0:t0 + QCH], ysb[:])

            # ---- filler machinery: drip units into the PE stream with a
            # per-k-tile credit budget (cost ~ matmul count)
            chunk_q = []   # (chunk_id, cost, thunk) -- deadline-ordered
            proj_q = []
            credit = [0.0]

            def emit_fill(budget):
                credit[0] += budget
                while True:
                    if chunk_q:
                        cid, cost, u = chunk_q[0]
                    elif proj_q:
                        cid, cost, u = 0, 1, None
                    else:
                        return
                    if cost > credit[0]:
                        return
                    credit[0] -= cost
                    if u is None:
                        proj_q.pop(0)()
                    else:
                        chunk_q.pop(0)
                        u()

            def force_chunks(upto):
                while chunk_q and chunk_q[0][0] < upto:
                    _, _, u = chunk_q.pop(0)
                    u()

            def attn(b, qc):
                t0 = b * T + qc * QCH
                otA = pot.tile([P, QCH], f32, tag="ot", name="otA")
                otB = pot.tile([P, QCH], f32, tag="ot", name="otB")
                pend = []

                def emit_av(kt, pt):
                    kti = b * NKT + kt
                    nc.tensor.matmul(
                        otA[0:HD + 1, :], vt_all[:, kti, 0:HD + 1],
                        pt[:, 0:QCH],
                        start=(kt == 0), stop=(kt == NKT - 1))
                    nc.tensor.matmul(
                        otB[0:HD + 1, :], vt_all[:, kti, VW:VW + HD + 1],
                        pt[:, QCH:2 * QCH],
                        start=(kt == 0), stop=(kt == NKT - 1))

                for kt in range(NKT):
                    k0 = b * T + kt * P
                    st = pst.tile([P, 2 * QCH], f32, tag="st", name="st")
                    nc.tensor.matmul(
                        st[:, 0:QCH], kt_all[0:HD, k0:k0 + P],
                        q_all[0:HD, t0:t0 + QCH], start=True, stop=True)
                    nc.tensor.matmul(
                        st[:, QCH:2 * QCH], kt_all[HD:P, k0:k0 + P],
                        q_all[HD:P, t0:t0 + QCH], start=True, stop=True)
                    pt = ptpool.tile([P, 2 * QCH], bf16, tag="pt", name="pt")
                    nc.scalar.activation(pt[:], st[:], AF.Exp, scale=SCALE)
                    pend.append((kt, pt))
                    if len(pend) > 4:
                        emit_av(*pend.pop(0))
                    emit_fill(4)
                if b == 0 and qc == 0:
                    force_chunks(2)   # chunk-1 k/v must precede the flush
                for pe_ in pend:
                    emit_av(*pe_)
                emit_fill(6)

                # normalize both heads into ot_all (Z sits at ot[HD])
                for hh, ot in ((0, otA), (1, otB)):
                    zrow = zpool.tile([1, QCH], f32, tag="z", name="zrow",
                                      bufs=1)
                    nc.vector.tensor_copy(zrow[:], ot[HD:HD + 1, :])
                    rz = zpool.tile([1, QCH], f32, tag="rz", name="rz")
                    nc.vector.reciprocal_approx_fast(rz[:], zrow[:])
                    rb = rzbp.tile([HD, QCH], f32, tag="rzb", name="rb")
                    nc.gpsimd.partition_broadcast(rb[:], rz[:])
                    nc.vector.tensor_mul(
                        ot_all[hh * HD:(hh + 1) * HD, t0:t0 + QCH],
                        ot[0:HD, :], rb[:])

            # ---- schedule ----
            # Minimal serial prefix before attention: k/q/v of chunk 0 only
            # (group (0,0) ktiles 0-7).  Chunk 1's k and v and the later q
            # units drip in as the FIRST fills, ordered so each lands a few
            # ktiles before its first consumer (k(c1) by kt 8, v(c1) tile j
            # by AV kt 8+j which trails emission by 4-5 ktiles, q(c0,h1) by
            # group (0,1), q(c1,*) by groups (0,2)/(0,3)).
            for h in range(CCH // QCH):
                qk_unit(0, h, wk_sb, 1, kt_all)
            qk_unit(0, 0, wq_sb, 0, q_all)
            for tt in range(CCH // P):
                v_unit(0, tt)
            for h in range(CCH // QCH):
                chunk_q.append(
                    (1, 8, lambda h=h: qk_unit(1, h, wk_sb, 1, kt_all)))
            for tt in range(CCH // P // 2):
                chunk_q.append((1, 8, lambda tt=tt: v_unit(1, tt)))
            chunk_q.append((0, 8, lambda: qk_unit(0, 1, wq_sb, 0, q_all)))
            for tt in range(CCH // P // 2, CCH // P):
                chunk_q.append((1, 8, lambda tt=tt: v_unit(1, tt)))
            chunk_q.append((1, 8, lambda: qk_unit(1, 0, wq_sb, 0, q_all)))
            chunk_q.append((1, 8, lambda: qk_unit(1, 1, wq_sb, 0, q_all)))

            for c in range(2, NCH):
                chunk_q.append((c, 0, lambda c=c: load_x(c)))
                for h in range(CCH // QCH):
                    chunk_q.append(
                        (c, 8,
                         lambda c=c, h=h: qk_unit(c, h, wk_sb, 1, kt_all)))
                for tt in range(CCH // P):
                    chunk_q.append((c, 8, lambda c=c, tt=tt: v_unit(c, tt)))
                for h in range(CCH // QCH):
                    chunk_q.append(
                        (c, 8,
                         lambda c=c, h=h: qk_unit(c, h, wq_sb, 0, q_all)))

            for b in range(B):
                if b >= 1:
                    force_chunks(2 * (b + 1) if b < B - 1 else NCH)
                for qc in range(NQC):
                    attn(b, qc)
                    for jt in range(NFT):
                        proj_q.append(
                            lambda b=b, qc=qc, jt=jt: proj_unit(b, qc, jt))
                    emit_fill(4)
            while chunk_q or proj_q:
                emit_fill(8)

    nc.compile()
    return nc


def make_in_maps(inputs):
    """Host-side sharding: full inputs -> per-core input dicts."""
    import ml_dtypes
    bf = ml_dtypes.bfloat16

    x = np.asarray(inputs["x"], dtype=np.float32)
    w_qkv = np.asarray(inputs["w_qkv"], dtype=np.float32)
    b_qkv = np.asarray(inputs["b_qkv"], dtype=np.float32)
    w_proj = np.asarray(inputs["w_proj"], dtype=np.float32)

    def sbuf_layout(a):
        # [1024 x-dims, 128 my-dims] -> [128 part, 8*128] SBUF image
        return np.ascontiguousarray(
            a.reshape(NFT, P, P).transpose(1, 0, 2).reshape(P, D)).astype(bf)

    xT = np.ascontiguousarray(x.reshape(NTOK, D).T).astype(bf)
    in_maps = []
    for i in range(NCORES):
        s = slice(P * i, P * (i + 1))
        in_maps.append({
            "xT": xT,
            "w_q": sbuf_layout(w_qkv[0:D][s].T),
            "w_k": sbuf_layout(w_qkv[D:2 * D][s].T),
            "w_v": sbuf_layout(w_qkv[2 * D:3 * D][s].T),
            "w_p": np.ascontiguousarray(w_proj[:, s].T).astype(bf),
            "b_qk": np.ascontiguousarray(
                np.stack([b_qkv[0:D][s], b_qkv[D:2 * D][s]], axis=1)),
        })
    return in_maps


def assemble_output(results, inputs):
    x = np.asarray(inputs["x"])
    w_proj = np.asarray(inputs["w_proj"], dtype=np.float64)
    b_qkv = np.asarray(inputs["b_qkv"], dtype=np.float64)
    b_proj = np.asarray(inputs["b_proj"], dtype=np.float64)
    b_eff = b_proj + w_proj @ b_qkv[2 * D:3 * D]

    acc = np.zeros((D, NTOK), dtype=np.float32)
    for i in range(NCORES):
        acc += np.asarray(results[i]["out"], dtype=np.float32)
    y = acc.T.astype(np.float64) + b_eff[None, :]
    return y.reshape(x.shape).astype(np.float32)


def run(inputs, trace=False, **kw):
    from concourse.bass_utils import run_bass_kernel_spmd
    key = "full"
    if key not in _COMPILED:
        _COMPILED[key] = build()
    nc = _COMPILED[key]
    in_maps = make_in_maps(inputs)
    res = run_bass_kernel_spmd(nc, in_maps, core_ids=list(range(NCORES)),
                               trace=trace, **kw)
    return res


def kernel(**inputs) -> np.ndarray:
    res = run(inputs, trace=False)
    return assemble_output(res.results, inputs)

